# revision 30
# baseline (speedup 1.0000x reference)
"""Trainium2 Bass kernel for single-head attention with projections.
Exact reconstruction of the v2 configuration (measured 68.0us best-of-3).

See kernel.py docstring for the algorithm description.
"""

import sys

sys.path.insert(0, "/opt/trn_rl_repo")

import numpy as np
import ml_dtypes

B, S, D, DN = 4, 2048, 1024, 64
SH = S // 2
NC = 8
DT = D // 128
SKT = S // 128

BF16 = np.dtype(ml_dtypes.bfloat16)
F8 = np.dtype(ml_dtypes.float8_e4m3)

ORDER = [0, 4, 1, 5, 2, 6, 3, 7, 8, 12, 9, 13, 10, 14, 11, 15]

PRE_FILL = 12
FILL_A = [1, 1, 1, 1, 1, 1, 0, 0]
FILL_B = [2, 2, 2, 2, 1, 1, 1, 0]

_prog = None


def _build_program():
    from concourse import tile, mybir, bacc

    f32 = mybir.dt.float32
    bf16 = mybir.dt.bfloat16
    f8 = mybir.dt.float8e4
    Exp = mybir.ActivationFunctionType.Exp
    MULT = mybir.AluOpType.mult

    nc = bacc.Bacc("TRN2", target_bir_lowering=False, num_devices=NC)

    kTz = nc.dram_tensor("kTz", [128, DT, S], f8, kind="ExternalInput")
    vTz = nc.dram_tensor("vTz", [128, DT, S], bf16, kind="ExternalInput")
    qTz = nc.dram_tensor("qTz", [128, DT, SH], f8, kind="ExternalInput")
    eTz = nc.dram_tensor("eTz", [128, SKT, SH], f8, kind="ExternalInput")
    ws8 = nc.dram_tensor("ws8", [128, DT, 2, DN], f8, kind="ExternalInput")
    wsv = nc.dram_tensor("wsv", [128, DT, DN], bf16, kind="ExternalInput")
    idb = nc.dram_tensor("idb", [128, DN], bf16, kind="ExternalInput")
    idf = nc.dram_tensor("idf", [65, 65], f32, kind="ExternalInput")
    outz = nc.dram_tensor("outz", [128, SH // 128, DN], bf16,
                          kind="ExternalOutput")

    with tile.TileContext(nc) as tc:
        with (
            tc.tile_pool(name="singles", bufs=1) as singles,
            tc.tile_pool(name="kio", bufs=DT // 2) as kio,
            tc.tile_pool(name="qio", bufs=DT // 2) as qio,
            tc.tile_pool(name="vio", bufs=DT // 2) as vio,
        ):
            w8_sb = singles.tile([128, DT, 2, DN], f8, tag="w8")
            nc.scalar.dma_start(w8_sb[:], ws8[:, :, :, :])
            wv_sb = singles.tile([128, DT, DN], bf16, tag="wv")
            nc.scalar.dma_start(wv_sb[:], wsv[:, :, :])

            kpT2 = singles.tile([128, S // 2], bf16, tag="kpT")
            vpT2 = singles.tile([128, S // 2], bf16, tag="vpT")
            qpT_d = singles.tile([128, SH], bf16, tag="qpT")
            vp_sb = singles.tile([128, SKT, DN + 1], bf16, tag="vp")
            nc.vector.memset(vp_sb[:, :, DN:DN + 1], 1.0)
            e_sb = singles.tile([128, SKT, SH], f8, tag="e")
            ident_d = singles.tile([128, DN], bf16, tag="idb")
            ident_f = singles.tile([65, 65], f32, tag="idf")
            av_sb = singles.tile([65, SH], f32, tag="avsb")
            ob2 = singles.tile([128, SH // 128, DN], bf16, tag="ob")
            fscr = singles.tile([128, 256], bf16, tag="fscr")
            nc.vector.memset(fscr[:], 0.0)
            fscr2 = singles.tile([128, 256], bf16, tag="fscr2")
            nc.vector.memset(fscr2[:], 0.0)

            kts, qts, vts = [], [], []

            with tc.tile_pool(name="pps", bufs=1, space="PSUM") as pps:
                kp_ps = [pps.tile([128, 512], f32, tag=f"kp{i}", name=f"kp{i}")
                         for i in range(2)]
                qp_ps = [pps.tile([128, 512], f32, tag=f"qp{i}", name=f"qp{i}")
                        for i in range(2)]
                fill_ps = pps.tile([128, 512], f32, tag="fil", name="fil")

                def filler(n):
                    for _ in range(n):
                        nc.tensor.matmul(fill_ps[0:32, 0:256], fscr[:, 0:32],
                                         fscr[:, 0:256], start=True, stop=True)

                def filler_wide(n):
                    # full 128-wide stationary: the HAM clock-gate does not
                    # register M=32 fillers as PE-busy; M=128 warms it
                    for _ in range(n):
                        nc.tensor.matmul(fill_ps[:, 0:256], fscr2[:, 0:128],
                                         fscr2[:, 0:256], start=True, stop=True)

                for tt in range(DT // 2):
                    kt = kio.tile([128, 2, S], f8, tag="kT", name=f"kt{tt}")
                    nc.sync.dma_start(kt[:], kTz[:, 2 * tt:2 * tt + 2, :])
                    kts.append(kt)
                    qt = qio.tile([128, 2, SH], f8, tag="qT", name=f"qt{tt}")
                    nc.scalar.dma_start(qt[:], qTz[:, 2 * tt:2 * tt + 2, :])
                    qts.append(qt)

                filler_wide(PRE_FILL)

                for t in range(DT):
                    kt = kts[t // 2][:, t % 2, :]
                    qt = qts[t // 2][:, t % 2, :]
                    st = dict(start=(t == 0), stop=(t == DT - 1))
                    for c in range(4):
                        nc.tensor.matmul(
                            kp_ps[c // 2][(c % 2) * 64:(c % 2) * 64 + 64, :],
                            w8_sb[:, t, 0, :], kt[:, c * 512:(c + 1) * 512],
                            tile_position=(0, (c % 2) * 64),
                            skip_group_check=(c % 2 == 1), **st)
                    filler(FILL_A[t])
                    for i in range(2):
                        cs = slice(i * 512, (i + 1) * 512)
                        nc.tensor.matmul(qp_ps[i][0:64, :], w8_sb[:, t, 1, :],
                                         qt[:, cs], tile_position=(0, 0), **st)
                        nc.tensor.matmul(qp_ps[i][64:128, :], w8_sb[:, t, 1, :],
                                         qt[:, cs], tile_position=(0, 64),
                                         skip_group_check=True, **st)
                    filler(FILL_B[t])

                for j4 in range(4):
                    js = slice(4 * j4, 4 * (j4 + 1))
                    nc.sync.dma_start(e_sb[:, js, :], eTz[:, js, :])
                nc.sync.dma_start(ident_d[:], idb[:, :])
                nc.sync.dma_start(ident_f[:], idf[:, :])

                for tt in range(DT // 2):
                    vt = vio.tile([128, 2, S], bf16, tag="vT", name=f"vt{tt}")
                    nc.scalar.dma_start(vt[:], vTz[:, 2 * tt:2 * tt + 2, :])
                    vts.append(vt)

                for i in range(2):
                    nc.any.tensor_copy(kpT2[:, i * 512:(i + 1) * 512], kp_ps[i])
                    nc.any.tensor_copy(qpT_d[:, i * 512:(i + 1) * 512],
                                       qp_ps[i])

            with (
                tc.tile_pool(name="expp", bufs=3) as expp,
                tc.tile_pool(name="statp", bufs=4) as statp,
                tc.tile_pool(name="attnp", bufs=14) as attnp,
                tc.tile_pool(name="avp", bufs=1, space="PSUM") as avp,
                tc.tile_pool(name="vpp", bufs=1, space="PSUM") as vpp,
                tc.tile_pool(name="sps", bufs=2, space="PSUM") as sps,
            ):
                av_ps = [avp.tile([128, 512], f32, tag=f"av{c}", name=f"av{c}")
                         for c in range(2)]
                vp_ps = [vpp.tile([128, 512], f32, tag=f"vq{p}", name=f"vp{p}")
                         for p in range(2)]

                def vp_proj_t(t):
                    st = dict(start=(t == 0), stop=(t == DT - 1))
                    for p in range(2):
                        for ci, c in enumerate((2 * p, 2 * p + 1)):
                            nc.tensor.matmul(
                                vp_ps[p][(c % 2) * 64:(c % 2) * 64 + 64, :],
                                wv_sb[:, t, :],
                                vts[t // 2][:, t % 2, c * 512:(c + 1) * 512],
                                tile_position=(0, (c % 2) * 64),
                                skip_group_check=(ci == 1), **st)

                def reorient_pair(p):
                    tp = vp_ps[p // 4][:, (p % 4) * 128:(p % 4) * 128 + 128]
                    for s in range(2):
                        j = 2 * p + s
                        c = j // 4
                        h = (c % 2) * 64
                        kc = (c // 2) * 512 + (j % 4) * 128
                        nc.tensor.matmul(tp[:, s * 64:(s + 1) * 64],
                                         vpT2[h:h + 64, kc:kc + 128],
                                         ident_d[h:h + 64, :],
                                         start=True, stop=True,
                                         skip_group_check=(s == 1))
                    nc.vector.tensor_copy(
                        vp_sb[:, 2 * p:2 * p + 2, 0:DN],
                        tp.rearrange("p (s n) -> p s n", s=2))

                def av_mm(j, idx, at):
                    for c in range(2):
                        nc.tensor.matmul(av_ps[c][0:65, :], vp_sb[:, j, 0:DN + 1],
                                         at[:, c * 512:(c + 1) * 512],
                                         start=(idx == 0), stop=(idx == SKT - 1))

                pend = []
                for idx, j in enumerate(ORDER):
                    c = j // 4
                    h = (c % 2) * 64
                    kc = (c // 2) * 512 + (j % 4) * 128
                    lhsT = kpT2[h:h + 64, kc:kc + 128]
                    ex = expp.tile([128, SH], bf16, tag="ex", name="ex")
                    at = attnp.tile([128, SH], bf16, tag="at", name="at")
                    sc = sps.tile([128, SH], f32, tag="sc", name="sc")
                    for i in range(2):
                        cs = slice(i * 512, (i + 1) * 512)
                        nc.tensor.matmul(sc[:, cs], lhsT, qpT_d[h:h + 64, cs],
                                         start=True, stop=True)
                    nc.scalar.activation(ex[:], sc[:], Exp, scale=1.0 / 8192.0)
                    nc.vector.tensor_tensor(at[:], ex[:], e_sb[:, idx, :], MULT)
                    if 2 <= idx <= 9:
                        vp_proj_t(idx - 2)
                    if idx == 10:
                        for p in range(2):
                            nc.vector.tensor_copy(
                                vpT2[:, p * 512:(p + 1) * 512], vp_ps[p])
                    if 10 <= idx <= 13:
                        for p in (2 * (idx - 10), 2 * (idx - 10) + 1):
                            reorient_pair(p)
                    pend.append((j, idx, at))
                    if idx >= 12:
                        av_mm(*pend.pop(0))
                        av_mm(*pend.pop(0))
                for c in range(2):
                    for p in pend:
                        j, idx, at = p
                        nc.tensor.matmul(av_ps[c][0:65, :],
                                         vp_sb[:, j, 0:DN + 1],
                                         at[:, c * 512:(c + 1) * 512],
                                         start=(idx == 0), stop=(idx == SKT - 1))
                    nc.vector.tensor_copy(av_sb[:, c * 512:(c + 1) * 512],
                                          av_ps[c][0:65, :])
                    for i in range(4 * c, 4 * c + 4):
                        tp = sps.tile([128, SH], f32, tag="sc", name="ot")
                        nc.tensor.transpose(tp[:, 0:65],
                                            av_sb[:, i * 128:(i + 1) * 128],
                                            ident_f[:, :])
                        recip = statp.tile([128, 1], f32, tag="recip")
                        nc.vector.reciprocal(recip, tp[:, DN:DN + 1])
                        nc.vector.tensor_scalar(ob2[:, i, :], tp[:, 0:DN],
                                                recip, None, MULT)
                    nc.sync.dma_start(outz[:, 4 * c:4 * c + 4, :],
                                      ob2[:, 4 * c:4 * c + 4, :])

    nc.finalize()
    return nc


def _get_program():
    global _prog
    if _prog is None:
        _prog = _build_program()
    return _prog


def _make_in_maps(q, k, v, mask, w_q, w_k, w_v):
    q = np.asarray(q, dtype=np.float32)
    k = np.asarray(k, dtype=np.float32)
    v = np.asarray(v, dtype=np.float32)
    mask = np.asarray(mask, dtype=np.float32)

    w8D = np.stack([
        np.asarray(w_k, np.float32).T * np.float32(32.0),
        np.asarray(w_q, np.float32).T * np.float32(32.0),
    ], axis=1)
    ws8 = np.ascontiguousarray(
        w8D.reshape(DT, 128, 2, DN).transpose(1, 0, 2, 3)).astype(F8)
    wsv = np.ascontiguousarray(
        np.asarray(w_v, np.float32).T.reshape(DT, 128, DN)
        .transpose(1, 0, 2)).astype(BF16)
    idb = np.concatenate([np.eye(DN, dtype=np.float32)] * 2, axis=0).astype(BF16)
    idf = np.eye(65, dtype=np.float32)

    kTzs = [np.ascontiguousarray(
        k[b].T.reshape(DT, 128, S).transpose(1, 0, 2)).astype(F8)
        for b in range(B)]
    vTzs = [np.ascontiguousarray(
        v[b].T.reshape(DT, 128, S).transpose(1, 0, 2)).astype(BF16)
        for b in range(B)]

    in_maps = []
    for c in range(NC):
        b, h = divmod(c, 2)
        sl = slice(h * SH, (h + 1) * SH)
        m = mask[b, sl, :]
        d = (m - m.min(axis=1, keepdims=True)) * np.float32(-1e9)
        with np.errstate(under="ignore"):
            e = np.exp(d, dtype=np.float32)
        eTz = np.ascontiguousarray(
            e.T.reshape(SKT, 128, SH)[ORDER].transpose(1, 0, 2)).astype(F8)
        in_maps.append({
            "kTz": kTzs[b],
            "vTz": vTzs[b],
            "qTz": np.ascontiguousarray(
                q[b, sl, :].T.reshape(DT, 128, SH).transpose(1, 0, 2)
            ).astype(F8),
            "eTz": eTz,
            "ws8": ws8,
            "wsv": wsv,
            "idb": idb,
            "idf": idf,
        })
    return in_maps


def _assemble_out(results):
    out = np.empty((B, S, DN), dtype=np.float32)
    for c in range(NC):
        b, h = divmod(c, 2)
        o = results[c]["outz"].astype(np.float32).transpose(1, 0, 2).reshape(SH, DN)
        out[b, h * SH:(h + 1) * SH, :] = o
    return out


def kernel(q, k, v, mask, w_q, b_q, w_k, b_k, w_v, b_v):
    from concourse import bass_utils

    in_maps = _make_in_maps(q, k, v, mask, w_q, w_k, w_v)
    nc = _get_program()
    res = bass_utils.run_bass_kernel_spmd(nc, in_maps, core_ids=list(range(NC)))
    return _assemble_out(res.results)


# revision 32
# speedup vs baseline: 1.0300x; 1.0300x over previous
"""Trainium2 Bass kernel for single-head attention with projections.

Reference (B=4, S=2048, D=1024, d_n=64, fp32): qp/kp/vp = x @ w.T;
scores = (qp @ kp.T)/8 + mask*(-1e9); out = softmax(scores) @ vp.

Sharding: 8 cores = 4 batches x 2 query halves; each core computes the
full K/V projections locally (cheaper than the pair-exchange collective,
which measured ~17us of critical-path latency).

Key structure (hardware-profile-driven; measured 69.1us best-of-3 at
rel err 2.9e-3 vs the 86.4us session baseline):
  * scores computed TRANSPOSED (scT[k,q] = kp @ qp^T) so attn^T is
    directly the AV matmul's moving operand -- no attention transposes.
  * mask folds host-side into E = exp(-1e9*(mask - rowmin)); device
    softmax is exp(scores)*E; denominator comes free from a ones-column
    in the AV matmul (M=65). E ships fp8 in consumption (ORDER) order.
  * k/q and weights ship fp8-e4m3 (weights pre-scaled x32; the combined
    scale folds into the exp's free scale operand); v/attn bf16; PSUM
    fp32; output bf16 (host upcasts).
  * all k/q/v tiles ship as two-tile DMAs and stay SBUF-resident (no
    ring-buffer trigger stalls). Sync ring: k -> E -> out; scalar ring:
    weights -> q -> v.
  * keep-warm filler matmuls pad the initial DMA-wait so the PE clock
    gate (HAM) ramps; v projections run t-outer into two PSUM banks
    interleaved at attention idx 2..9; reorientation uses the then-dead
    vp psum banks as scratch; AV matmuls drain 2/idx from idx 12 with
    per-chunk finalization and split output DMAs.

Measured-and-rejected variants (all slower on this DMA system): SWDGE
fp8->bf16 cast-loads for E (ring contention), k split across both HWDGE
rings, v gated behind k, single-tile k DMAs, copies on the scalar queue
(sem-lane-reuse blocking), earlier/denser AV pops.
"""

import sys

sys.path.insert(0, "/opt/trn_rl_repo")

import numpy as np
import ml_dtypes

B, S, D, DN = 4, 2048, 1024, 64
SH = S // 2
NC = 8
DT = D // 128
SKT = S // 128

BF16 = np.dtype(ml_dtypes.bfloat16)
F8 = np.dtype(ml_dtypes.float8_e4m3)

ORDER = [0, 4, 1, 5, 2, 6, 3, 7, 8, 12, 9, 13, 10, 14, 11, 15]

PRE_FILL = 14
FILL_A = [1, 1, 1, 1, 1, 1, 0, 0]
FILL_B = [2, 2, 2, 2, 1, 1, 1, 0]

_prog = None


def _build_program():
    from concourse import tile, mybir, bacc

    f32 = mybir.dt.float32
    bf16 = mybir.dt.bfloat16
    f8 = mybir.dt.float8e4
    Exp = mybir.ActivationFunctionType.Exp
    MULT = mybir.AluOpType.mult

    nc = bacc.Bacc("TRN2", target_bir_lowering=False, num_devices=NC)

    kTz = nc.dram_tensor("kTz", [128, 2, DT, S // 2], f8, kind="ExternalInput")
    vTz = nc.dram_tensor("vTz", [128, DT, S], bf16, kind="ExternalInput")
    qTz = nc.dram_tensor("qTz", [128, DT, SH], f8, kind="ExternalInput")
    eTz = nc.dram_tensor("eTz", [128, SKT, SH], f8, kind="ExternalInput")
    ws8 = nc.dram_tensor("ws8", [128, DT, 2, DN], f8, kind="ExternalInput")
    wsv = nc.dram_tensor("wsv", [128, DT, DN], bf16, kind="ExternalInput")
    idb = nc.dram_tensor("idb", [128, DN], bf16, kind="ExternalInput")
    idf = nc.dram_tensor("idf", [65, 65], f32, kind="ExternalInput")
    outz = nc.dram_tensor("outz", [128, SH // 128, DN], bf16,
                          kind="ExternalOutput")

    with tile.TileContext(nc) as tc:
        with (
            tc.tile_pool(name="singles", bufs=1) as singles,
            tc.tile_pool(name="kio", bufs=DT // 2) as kio,
            tc.tile_pool(name="qio", bufs=DT // 2) as qio,
            tc.tile_pool(name="vio", bufs=DT // 2) as vio,
        ):
            w8_sb = singles.tile([128, DT, 2, DN], f8, tag="w8")
            nc.scalar.dma_start(w8_sb[:], ws8[:, :, :, :])
            wv_sb = singles.tile([128, DT, DN], bf16, tag="wv")
            nc.scalar.dma_start(wv_sb[:], wsv[:, :, :])

            kpT2 = singles.tile([128, S // 2], bf16, tag="kpT")
            vpT2 = singles.tile([128, S // 2], bf16, tag="vpT")
            qpT_d = singles.tile([128, SH], bf16, tag="qpT")
            vp_sb = singles.tile([128, SKT, DN + 1], bf16, tag="vp")
            nc.vector.memset(vp_sb[:, :, DN:DN + 1], 1.0)
            e_sb = singles.tile([128, SKT, SH], f8, tag="e")
            ident_d = singles.tile([128, DN], bf16, tag="idb")
            ident_f = singles.tile([65, 65], f32, tag="idf")
            av_sb = singles.tile([65, SH], f32, tag="avsb")
            ob2 = singles.tile([128, SH // 128, DN], bf16, tag="ob")
            fscr = singles.tile([128, 256], bf16, tag="fscr")
            nc.vector.memset(fscr[:], 0.0)
            fscr2 = singles.tile([128, 256], bf16, tag="fscr2")
            nc.vector.memset(fscr2[:], 0.0)

            kts, qts, vts = [], [], []

            with tc.tile_pool(name="pps", bufs=1, space="PSUM") as pps:
                kp_ps = [pps.tile([128, 512], f32, tag="kp0", name="kp0")]
                qp_ps = [pps.tile([128, 512], f32, tag=f"qp{i}", name=f"qp{i}")
                        for i in range(2)]
                fill_ps = pps.tile([128, 512], f32, tag="fil", name="fil")

                def filler(n):
                    for _ in range(n):
                        nc.tensor.matmul(fill_ps[0:32, 0:256], fscr[:, 0:32],
                                         fscr[:, 0:256], start=True, stop=True)

                def filler_wide(n):
                    # full-width: the HAM clock-gate ignores M=32 fillers
                    for _ in range(n):
                        nc.tensor.matmul(fill_ps[:, 0:256], fscr2[:, 0:128],
                                         fscr2[:, 0:256], start=True, stop=True)

                # k ships by S-HALVES: attention consumes j 0-7 (= keys
                # 0-1023 = kp bank 0) first, so scores/exp start after only
                # half of k has landed. E chunk 0 rides between the halves.
                for h in range(2):
                    kt = kio.tile([128, DT, S // 2], f8, tag="kT",
                                  name=f"kh{h}")
                    nc.sync.dma_start(kt[:], kTz[:, h, :, :])
                    kts.append(kt)
                    if h == 0:
                        nc.sync.dma_start(e_sb[:, 0:4, :], eTz[:, 0:4, :])
                for tt in range(DT // 2):
                    qt = qio.tile([128, 2, SH], f8, tag="qT", name=f"qt{tt}")
                    nc.scalar.dma_start(qt[:], qTz[:, 2 * tt:2 * tt + 2, :])
                    qts.append(qt)

                filler_wide(PRE_FILL)

                for t in range(DT):
                    kt = kts[0][:, t, :]
                    qt = qts[t // 2][:, t % 2, :]
                    st = dict(start=(t == 0), stop=(t == DT - 1))
                    # kp S-half 0: chunks 0,1 -> kp bank 0, halves h0/h64
                    for c in range(2):
                        nc.tensor.matmul(
                            kp_ps[0][(c % 2) * 64:(c % 2) * 64 + 64, :],
                            w8_sb[:, t, 0, :], kt[:, c * 512:(c + 1) * 512],
                            tile_position=(0, (c % 2) * 64),
                            skip_group_check=(c % 2 == 1), **st)
                    filler(FILL_A[t])
                    for i in range(2):
                        cs = slice(i * 512, (i + 1) * 512)
                        nc.tensor.matmul(qp_ps[i][0:64, :], w8_sb[:, t, 1, :],
                                         qt[:, cs], tile_position=(0, 0), **st)
                        nc.tensor.matmul(qp_ps[i][64:128, :], w8_sb[:, t, 1, :],
                                         qt[:, cs], tile_position=(0, 64),
                                         skip_group_check=True, **st)
                    filler(FILL_B[t])

                for j4 in range(1, 4):
                    js = slice(4 * j4, 4 * (j4 + 1))
                    nc.sync.dma_start(e_sb[:, js, :], eTz[:, js, :])
                nc.sync.dma_start(ident_d[:], idb[:, :])
                nc.sync.dma_start(ident_f[:], idf[:, :])

                for tt in range(DT // 2):
                    vt = vio.tile([128, 2, S], bf16, tag="vT", name=f"vt{tt}")
                    nc.scalar.dma_start(vt[:], vTz[:, 2 * tt:2 * tt + 2, :])
                    vts.append(vt)

                nc.vector.tensor_copy(kpT2[:, 0:512], kp_ps[0])
                for i in range(2):
                    nc.vector.tensor_copy(qpT_d[:, i * 512:(i + 1) * 512],
                                          qp_ps[i])

            with (
                tc.tile_pool(name="expp", bufs=3) as expp,
                tc.tile_pool(name="statp", bufs=4) as statp,
                tc.tile_pool(name="attnp", bufs=14) as attnp,
                tc.tile_pool(name="avp", bufs=1, space="PSUM") as avp,
                tc.tile_pool(name="vpp", bufs=1, space="PSUM") as vpp,
                tc.tile_pool(name="sps", bufs=2, space="PSUM") as sps,
            ):
                av_ps = [avp.tile([128, 512], f32, tag=f"av{c}", name=f"av{c}")
                         for c in range(2)]
                vp_ps = [vpp.tile([128, 512], f32, tag=f"vq{p}", name=f"vp{p}")
                         for p in range(2)]

                def vp_proj_t(t):
                    st = dict(start=(t == 0), stop=(t == DT - 1))
                    for p in range(2):
                        for ci, c in enumerate((2 * p, 2 * p + 1)):
                            nc.tensor.matmul(
                                vp_ps[p][(c % 2) * 64:(c % 2) * 64 + 64, :],
                                wv_sb[:, t, :],
                                vts[t // 2][:, t % 2, c * 512:(c + 1) * 512],
                                tile_position=(0, (c % 2) * 64),
                                skip_group_check=(ci == 1), **st)

                def reorient_pair(p):
                    tp = vp_ps[p // 4][:, (p % 4) * 128:(p % 4) * 128 + 128]
                    for s in range(2):
                        j = 2 * p + s
                        c = j // 4
                        h = (c % 2) * 64
                        kc = (c // 2) * 512 + (j % 4) * 128
                        nc.tensor.matmul(tp[:, s * 64:(s + 1) * 64],
                                         vpT2[h:h + 64, kc:kc + 128],
                                         ident_d[h:h + 64, :],
                                         start=True, stop=True,
                                         skip_group_check=(s == 1))
                    nc.vector.tensor_copy(
                        vp_sb[:, 2 * p:2 * p + 2, 0:DN],
                        tp.rearrange("p (s n) -> p s n", s=2))

                def av_mm(j, idx, at):
                    for c in range(2):
                        nc.tensor.matmul(av_ps[c][0:65, :], vp_sb[:, j, 0:DN + 1],
                                         at[:, c * 512:(c + 1) * 512],
                                         start=(idx == 0), stop=(idx == SKT - 1))

                def kp_half1(i4):
                    # d-tiles 2*i4, 2*i4+1 of the k S-half-1 projection,
                    # accumulated in av_ps[0] (idle until the idx-12 AV
                    # pops, whose start=True clears it)
                    for t in (2 * i4, 2 * i4 + 1):
                        st = dict(start=(t == 0), stop=(t == DT - 1))
                        for c in range(2):
                            nc.tensor.matmul(
                                av_ps[0][(c % 2) * 64:(c % 2) * 64 + 64, :],
                                w8_sb[:, t, 0, :],
                                kts[1][:, t, c * 512:(c + 1) * 512],
                                tile_position=(0, (c % 2) * 64),
                                skip_group_check=(c % 2 == 1), **st)

                pend = []
                for idx, j in enumerate(ORDER):
                    c = j // 4
                    h = (c % 2) * 64
                    kc = (c // 2) * 512 + (j % 4) * 128
                    lhsT = kpT2[h:h + 64, kc:kc + 128]
                    ex = expp.tile([128, SH], bf16, tag="ex", name="ex")
                    at = attnp.tile([128, SH], bf16, tag="at", name="at")
                    sc = sps.tile([128, SH], f32, tag="sc", name="sc")
                    for i in range(2):
                        cs = slice(i * 512, (i + 1) * 512)
                        nc.tensor.matmul(sc[:, cs], lhsT, qpT_d[h:h + 64, cs],
                                         start=True, stop=True)
                    nc.scalar.activation(ex[:], sc[:], Exp, scale=1.0 / 8192.0)
                    nc.vector.tensor_tensor(at[:], ex[:], e_sb[:, idx, :], MULT)
                    if idx <= 3:
                        kp_half1(idx)
                    if idx == 4:
                        nc.vector.tensor_copy(kpT2[:, 512:1024], av_ps[0])
                    if 2 <= idx <= 9:
                        vp_proj_t(idx - 2)
                    if idx == 10:
                        for p in range(2):
                            nc.vector.tensor_copy(
                                vpT2[:, p * 512:(p + 1) * 512], vp_ps[p])
                    if 10 <= idx <= 13:
                        for p in (2 * (idx - 10), 2 * (idx - 10) + 1):
                            reorient_pair(p)
                    pend.append((j, idx, at))
                    if idx >= 12:
                        av_mm(*pend.pop(0))
                        av_mm(*pend.pop(0))
                for c in range(2):
                    for p in pend:
                        j, idx, at = p
                        nc.tensor.matmul(av_ps[c][0:65, :],
                                         vp_sb[:, j, 0:DN + 1],
                                         at[:, c * 512:(c + 1) * 512],
                                         start=(idx == 0), stop=(idx == SKT - 1))
                    nc.vector.tensor_copy(av_sb[:, c * 512:(c + 1) * 512],
                                          av_ps[c][0:65, :])
                    for i in range(4 * c, 4 * c + 4):
                        tp = sps.tile([128, SH], f32, tag="sc", name="ot")
                        nc.tensor.transpose(tp[:, 0:65],
                                            av_sb[:, i * 128:(i + 1) * 128],
                                            ident_f[:, :])
                        recip = statp.tile([128, 1], f32, tag="recip")
                        nc.vector.reciprocal(recip, tp[:, DN:DN + 1])
                        nc.vector.tensor_scalar(ob2[:, i, :], tp[:, 0:DN],
                                                recip, None, MULT)
                    nc.sync.dma_start(outz[:, 4 * c:4 * c + 4, :],
                                      ob2[:, 4 * c:4 * c + 4, :])

    nc.finalize()
    return nc


def _get_program():
    global _prog
    if _prog is None:
        _prog = _build_program()
    return _prog


def _make_in_maps(q, k, v, mask, w_q, w_k, w_v):
    q = np.asarray(q, dtype=np.float32)
    k = np.asarray(k, dtype=np.float32)
    v = np.asarray(v, dtype=np.float32)
    mask = np.asarray(mask, dtype=np.float32)

    w8D = np.stack([
        np.asarray(w_k, np.float32).T * np.float32(32.0),
        np.asarray(w_q, np.float32).T * np.float32(32.0),
    ], axis=1)
    ws8 = np.ascontiguousarray(
        w8D.reshape(DT, 128, 2, DN).transpose(1, 0, 2, 3)).astype(F8)
    wsv = np.ascontiguousarray(
        np.asarray(w_v, np.float32).T.reshape(DT, 128, DN)
        .transpose(1, 0, 2)).astype(BF16)
    idb = np.concatenate([np.eye(DN, dtype=np.float32)] * 2, axis=0).astype(BF16)
    idf = np.eye(65, dtype=np.float32)

    # k packed by S-halves: kTz[p, h, t, s] = k[b].T[t*128+p, h*1024+s]
    kTzs = [np.ascontiguousarray(
        k[b].T.reshape(DT, 128, 2, S // 2).transpose(1, 2, 0, 3)).astype(F8)
        for b in range(B)]
    vTzs = [np.ascontiguousarray(
        v[b].T.reshape(DT, 128, S).transpose(1, 0, 2)).astype(BF16)
        for b in range(B)]

    in_maps = []
    for c in range(NC):
        b, h = divmod(c, 2)
        sl = slice(h * SH, (h + 1) * SH)
        m = mask[b, sl, :]
        d = (m - m.min(axis=1, keepdims=True)) * np.float32(-1e9)
        with np.errstate(under="ignore"):
            e = np.exp(d, dtype=np.float32)
        eTz = np.ascontiguousarray(
            e.T.reshape(SKT, 128, SH)[ORDER].transpose(1, 0, 2)).astype(F8)
        in_maps.append({
            "kTz": kTzs[b],
            "vTz": vTzs[b],
            "qTz": np.ascontiguousarray(
                q[b, sl, :].T.reshape(DT, 128, SH).transpose(1, 0, 2)
            ).astype(F8),
            "eTz": eTz,
            "ws8": ws8,
            "wsv": wsv,
            "idb": idb,
            "idf": idf,
        })
    return in_maps


def _assemble_out(results):
    out = np.empty((B, S, DN), dtype=np.float32)
    for c in range(NC):
        b, h = divmod(c, 2)
        o = results[c]["outz"].astype(np.float32).transpose(1, 0, 2).reshape(SH, DN)
        out[b, h * SH:(h + 1) * SH, :] = o
    return out


def kernel(q, k, v, mask, w_q, b_q, w_k, b_k, w_v, b_v):
    from concourse import bass_utils

    in_maps = _make_in_maps(q, k, v, mask, w_q, w_k, w_v)
    nc = _get_program()
    res = bass_utils.run_bass_kernel_spmd(nc, in_maps, core_ids=list(range(NC)))
    return _assemble_out(res.results)
